# revision 26
# baseline (speedup 1.0000x reference)
"""Trainium2 Bass kernel for nn_CustomGCNLayer (GCN layer with dense
symmetric adjacency built from an edge list, set semantics).

  h   = x @ W.T + b_lin
  A   = symmetric 0/1 adjacency from edge_index (duplicates collapse)
  deg = A.sum(1);  dinv = (deg + 1e-6) ** -0.5
  out = dinv[:, None] * (A @ (dinv[:, None] * h)) + bias

Distribution over 8 NeuronCores (SPMD, fully independent cores - NO
collectives): column-shard the output. Core k owns output rows
R_k = [k*R, (k+1)*R) and computes

  out2[:, R_k] slice via  out2^T[d, i] = sum_j g[j, d] * A[j, i in R_k]

with g = dinv * (x @ W.T) computed REPLICATED on every core (the linear
layer is tiny: 64 matmuls of [128x128x128]).  The b_lin contribution is
algebraically folded out:  sum_j A_ij dinv_j (h_j) = out2_nb + c_i*b_lin
with c_i = sum_j A_ij dinv_j precomputed on host, so the GEMM runs on
g = dinv*(x@W.T) alone and the tail adds dinv_i*c_i*b_lin + bias.

Key device-side choices:
  - adjacency is shipped as a DENSE per-core [N, R] fp8e4 (0/1 exact)
    matrix, laid out in HBM exactly as the SBUF tiles consumed by the
    PE ([NG, 128, JGRP*R]); streamed by 1MB contiguous DMAs (~340GB/s)
    that fully overlap the GEMM.  No GPSIMD scatter, no collective.
  - main GEMM: 128 accumulating matmuls lhsT=g-block (bf16) x
    rhs=adjacency tile (fp8e4) -> PSUM [128, 1024] f32 held across the
    whole contraction; PE stays continuously busy (full 2.4GHz pstate).
  - tail: 8 PE transposes + fused dinv scale + (dinv*c)*b_lin + bias add.
"""

import dataclasses
import sys

import numpy as np
import ml_dtypes

if "/opt/trn_rl_repo" not in sys.path:
    sys.path.insert(0, "/opt/trn_rl_repo")

import concourse.bacc as bacc
import concourse.bass as bass
import concourse.mybir as mybir
import concourse.tile as tile
from concourse.masks import make_identity

F32 = mybir.dt.float32
BF16 = mybir.dt.bfloat16
FP8 = mybir.dt.float8e4
Alu = mybir.AluOpType

FP8_NP = ml_dtypes.float8_e4m3
BF16_NP = ml_dtypes.bfloat16


@dataclasses.dataclass(frozen=True)
class Cfg:
    N: int = 8192          # nodes
    D: int = 128           # features (in == out)
    C: int = 8             # cores
    JGRP: int = 4          # j-blocks (of 128) per adjacency DMA group

    @property
    def R(self):  # output rows per core
        return self.N // self.C

    @property
    def IB(self):  # 128-row output blocks per core
        return self.R // 128

    @property
    def JB(self):  # 128-row contraction blocks (all nodes)
        return self.N // 128

    @property
    def NG(self):  # adjacency DMA groups
        return self.JB // self.JGRP

    @property
    def XCH(self):  # x1T load chunks
        return 8


FULL = Cfg()


def build(cfg: Cfg) -> bass.Bass:
    N, D, C, R, IB, JB = cfg.N, cfg.D, cfg.C, cfg.R, cfg.IB, cfg.JB
    JGRP, NG = cfg.JGRP, cfg.NG
    JW = 512               # matmul free-dim chunk
    NCW = R // JW          # free-dim chunks per output (2)

    nc = bacc.Bacc()

    x1T = nc.dram_tensor("x1T", [D, N], BF16, kind="ExternalInput")
    WT = nc.dram_tensor("WT", [D, D], BF16, kind="ExternalInput")
    adj = nc.dram_tensor("adj", [NG, 128, JGRP * R], FP8, kind="ExternalInput")
    qb = nc.dram_tensor("qb", [128, IB * D], BF16, kind="ExternalInput")
    dinv_bc = nc.dram_tensor("dinv_bc", [128, R], BF16, kind="ExternalInput")
    out = nc.dram_tensor("out", [R, D], F32, kind="ExternalOutput")

    with tile.TileContext(nc, num_cores=C) as tc:
        const_p = tc.alloc_tile_pool(name="const", bufs=1)
        psA = tc.alloc_tile_pool(name="psA", bufs=1, space="PSUM")
        psB = tc.alloc_tile_pool(name="psB", bufs=5, space="PSUM")
        adj_p = tc.alloc_tile_pool(name="adjp", bufs=1)
        stage_p = tc.alloc_tile_pool(name="stage", bufs=2)

        # ---- inputs: ONE hwdge ring (sync), FIFO ordered by first use ---
        # WT first (gates every g-matmul), then x1T chunks, then the
        # adjacency stream (8MB, all resident: no slot waits), tail consts
        # last.
        WT_sb = const_p.tile([D, D], BF16, name="WT_sb")
        nc.sync.dma_start(out=WT_sb[:], in_=WT[:])
        x1T_sb = const_p.tile([D, N], BF16, name="x1T_sb")
        xw = N // cfg.XCH
        for q in range(cfg.XCH):
            nc.sync.dma_start(out=x1T_sb[:, q * xw:(q + 1) * xw],
                              in_=x1T[:, q * xw:(q + 1) * xw])
        at_tiles = []
        for jg in range(NG):
            at = adj_p.tile([128, JGRP * R], FP8, name=f"at{jg}")
            nc.sync.dma_start(out=at[:], in_=adj[jg, :, :])
            at_tiles.append(at)
        qb_sb = const_p.tile([128, IB * D], BF16, name="qb_sb")
        nc.sync.dma_start(out=qb_sb[:], in_=qb[:])
        dinv_sb = const_p.tile([128, R], BF16, name="dinv_sb")
        nc.sync.dma_start(out=dinv_sb[:], in_=dinv_bc[:])
        ident = const_p.tile([128, 128], F32, name="ident")
        make_identity(nc, ident[:])

        # ---- PE pstate warmup: dummy matmuls on scratch tiles while the
        # input DMAs land (PE reaches full clock after ~3us of activity) --
        warm = const_p.tile([128, 256], BF16, name="warm")
        nc.vector.memset(warm[:], 0.0)
        for w in range(10):
            ps_w = psB.tile([128, 512], F32, name="ps_w", tag="ps")
            nc.tensor.matmul(ps_w[:, 0:128], lhsT=warm[:, 0:128],
                             rhs=warm[:, 128:256], start=True, stop=True)

        # ---- g = dinv * (x @ W.T), node-major blocks [j, d] -------------
        # g[jb*128+p, d] = sum_k x1T[k, jb*128+p] * WT[k, d]
        g_sb = const_p.tile([128, N], BF16, name="g_sb")
        for q in range(JB // 4):
            ps_g = psB.tile([128, 512], F32, name="ps_g", tag="ps")
            for t in range(4):
                jb = q * 4 + t
                nc.tensor.matmul(ps_g[:, t * D:(t + 1) * D],
                                 lhsT=x1T_sb[:, jb * 128:(jb + 1) * 128],
                                 rhs=WT_sb[:], start=True, stop=True)
            dst = g_sb[:, q * 512:(q + 1) * 512]
            if q % 2 == 0:
                nc.scalar.copy(dst, ps_g[:])
            else:
                nc.vector.tensor_copy(dst, ps_g[:])

        # ---- main GEMM: out2^T[d, i] += g_jb^T A[jb block, own cols] ----
        ps_out = psA.tile([128, R], F32, name="ps_out", tag="po")
        for jg in range(NG):
            at = at_tiles[jg]
            for t in range(JGRP):
                jb = jg * JGRP + t
                for c in range(NCW):
                    nc.tensor.matmul(
                        ps_out[:, c * JW:(c + 1) * JW],
                        lhsT=g_sb[:, jb * 128:(jb + 1) * 128],
                        rhs=at[:, t * R + c * JW:t * R + (c + 1) * JW],
                        start=(jb == 0), stop=(jb == JB - 1))

        # ---- tail: dinv scale (pre-transpose, broadcast multiplier),
        #      transpose, fused psum->sbuf + qb add, store ----------------
        o2 = stage_p.tile([128, R], F32, name="o2")
        o2r = stage_p.tile([128, JW], F32, name="o2r")
        nc.vector.tensor_tensor(o2[:, 0:JW], ps_out[:, 0:JW],
                                dinv_sb[:, 0:JW], Alu.mult)
        nc.scalar.copy(o2r[:], ps_out[:, JW:R])  # GPSIMD has no PSUM port
        nc.gpsimd.tensor_tensor(o2[:, JW:R], o2r[:],
                                dinv_sb[:, JW:R], Alu.mult)
        st = stage_p.tile([128, IB * D], F32, name="st")
        for b in range(IB):
            ps_t = psB.tile([128, 512], F32, name="ps_t", tag="ps")
            nc.tensor.transpose(ps_t[:, 0:128], o2[:, b * 128:(b + 1) * 128],
                                ident[:])
            sb = st[:, b * D:(b + 1) * D]
            nc.vector.tensor_tensor(sb, ps_t[:, 0:128],
                                    qb_sb[:, b * D:(b + 1) * D], Alu.add)
            eng = nc.scalar if b % 2 == 0 else nc.sync
            eng.dma_start(out=out[b * 128:(b + 1) * 128, :], in_=sb)

        for p in [stage_p, adj_p, psB, psA, const_p]:
            p.release()

    return nc


def make_in_maps(cfg: Cfg, x, edge_index, W, b_lin, bias):
    N, D, C, R, IB = cfg.N, cfg.D, cfg.C, cfg.R, cfg.IB
    x = np.asarray(x, dtype=np.float32)
    W = np.asarray(W, dtype=np.float32)
    b_lin = np.asarray(b_lin, dtype=np.float32)
    bias = np.asarray(bias, dtype=np.float32)
    ei = np.asarray(edge_index[0]).astype(np.int64)
    ej = np.asarray(edge_index[1]).astype(np.int64)

    # unique symmetric (dest, col) pairs == reference's at[].set collapse
    key = np.unique(np.concatenate([ei * N + ej, ej * N + ei]))
    dest = (key // N).astype(np.int64)
    col = (key % N).astype(np.int64)

    deg = np.bincount(dest, minlength=N).astype(np.float32)
    dinv = ((deg + np.float32(1e-6)) ** -0.5).astype(np.float32)
    # c_i = sum_j A_ij * dinv_j  (b_lin propagation constant)
    c = np.bincount(dest, weights=dinv[col].astype(np.float64),
                    minlength=N).astype(np.float32)

    one_fp8 = np.float32(1.0).astype(FP8_NP).view(np.uint8)
    A_u8 = np.zeros((N, N), np.uint8)
    A_u8[dest, col] = one_fp8

    x1T = np.ascontiguousarray((dinv[:, None] * x).T).astype(BF16_NP)
    WT = np.ascontiguousarray(W.T).astype(BF16_NP)

    in_maps = []
    for k in range(C):
        own = slice(k * R, (k + 1) * R)
        adj_k = (A_u8[:, own]
                 .reshape(cfg.NG, cfg.JGRP, 128, R)
                 .transpose(0, 2, 1, 3)
                 .reshape(cfg.NG, 128, cfg.JGRP * R))
        dinv_o = dinv[own].reshape(IB, 128)
        q = (dinv_o * c[own].reshape(IB, 128))              # [IB, 128]
        qb = (q[:, :, None] * b_lin[None, None, :]
              + bias[None, None, :]).astype(np.float32)     # [IB, 128, D]
        in_maps.append({
            "x1T": x1T,
            "WT": WT,
            "adj": np.ascontiguousarray(adj_k).view(FP8_NP),
            "qb": np.ascontiguousarray(
                qb.transpose(1, 0, 2).reshape(128, IB * D)).astype(BF16_NP),
            "dinv_bc": np.ascontiguousarray(np.broadcast_to(
                dinv[own].astype(BF16_NP)[None, :], (128, R))),
        })
    return in_maps


def kernel(x, edge_index, W, b_lin, bias, *, trace=False, cfg: Cfg = FULL):
    from concourse.bass_utils import run_bass_kernel_spmd

    if trace:
        _install_ntff_hook()
    in_maps = make_in_maps(cfg, x, edge_index, W, b_lin, bias)
    nc = build(cfg)
    nc.finalize()
    res = run_bass_kernel_spmd(nc, in_maps, core_ids=list(range(cfg.C)),
                               trace=trace)
    full = np.concatenate([r["out"] for r in res.results], axis=0)
    kernel.last_results = res
    return full.astype(np.float32)


kernel.last_results = None


def _install_ntff_hook():
    """Provide antenv.axon_hooks (missing on this image) so that
    run_bass_kernel_spmd(trace=True) can capture NTFF profiles via the
    axon ctypes hook from trn_agent_boot."""
    import sys as _sys
    import types

    try:
        import antenv.axon_hooks  # noqa: F401
        return True
    except ImportError:
        pass
    try:
        import antenv
        from trn_agent_boot.trn_boot import _ntff_profile_via_ctypes

        hook = _ntff_profile_via_ctypes("/opt/axon/libaxon_pjrt.so")
        mod = types.ModuleType("antenv.axon_hooks")
        mod.get_axon_ntff_profile_hook = lambda: hook
        mod.set_axon_ntff_profile_hook = lambda h: None
        _sys.modules["antenv.axon_hooks"] = mod
        antenv.axon_hooks = mod
        return hook is not None
    except Exception as e:  # profiling is best-effort
        print(f"ntff hook install failed: {e}", file=sys.stderr)
        return False


# revision 27
# speedup vs baseline: 1.0452x; 1.0452x over previous
"""Trainium2 Bass kernel for nn_CustomGCNLayer (GCN layer with dense
symmetric adjacency built from an edge list, set semantics).

  h   = x @ W.T + b_lin
  A   = symmetric 0/1 adjacency from edge_index (duplicates collapse)
  deg = A.sum(1);  dinv = (deg + 1e-6) ** -0.5
  out = dinv[:, None] * (A @ (dinv[:, None] * h)) + bias

Distribution over 8 NeuronCores (SPMD, fully independent cores - NO
collectives): column-shard the output. Core k owns output rows
R_k = [k*R, (k+1)*R) and computes

  out2[:, R_k] slice via  out2^T[d, i] = sum_j g[j, d] * A[j, i in R_k]

with g = dinv * (x @ W.T) computed REPLICATED on every core (the linear
layer is tiny: 64 matmuls of [128x128x128]).  The b_lin contribution is
algebraically folded out:  sum_j A_ij dinv_j (h_j) = out2_nb + c_i*b_lin
with c_i = sum_j A_ij dinv_j precomputed on host, so the GEMM runs on
g = dinv*(x@W.T) alone and the tail adds dinv_i*c_i*b_lin + bias.

Key device-side choices:
  - adjacency is shipped as a DENSE per-core [N, R] fp8e4 (0/1 exact)
    matrix, laid out in HBM exactly as the SBUF tiles consumed by the
    PE ([NG, 128, JGRP*R]); streamed by 1MB contiguous DMAs (~340GB/s)
    that fully overlap the GEMM.  No GPSIMD scatter, no collective.
  - main GEMM: 128 accumulating matmuls lhsT=g-block (bf16) x
    rhs=adjacency tile (fp8e4) -> PSUM [128, 1024] f32 held across the
    whole contraction; PE stays continuously busy (full 2.4GHz pstate).
  - tail: 8 PE transposes + fused dinv scale + (dinv*c)*b_lin + bias add.
"""

import dataclasses
import sys

import numpy as np
import ml_dtypes

if "/opt/trn_rl_repo" not in sys.path:
    sys.path.insert(0, "/opt/trn_rl_repo")

import concourse.bacc as bacc
import concourse.bass as bass
import concourse.mybir as mybir
import concourse.tile as tile
from concourse.masks import make_identity

F32 = mybir.dt.float32
BF16 = mybir.dt.bfloat16
FP8 = mybir.dt.float8e4
Alu = mybir.AluOpType

FP8_NP = ml_dtypes.float8_e4m3
BF16_NP = ml_dtypes.bfloat16


@dataclasses.dataclass(frozen=True)
class Cfg:
    N: int = 8192          # nodes
    D: int = 128           # features (in == out)
    C: int = 8             # cores
    JGRP: int = 4          # j-blocks (of 128) per adjacency DMA group

    @property
    def R(self):  # output rows per core
        return self.N // self.C

    @property
    def IB(self):  # 128-row output blocks per core
        return self.R // 128

    @property
    def JB(self):  # 128-row contraction blocks (all nodes)
        return self.N // 128

    @property
    def NG(self):  # adjacency DMA groups
        return self.JB // self.JGRP

    @property
    def XCH(self):  # x1T load chunks
        return 8


FULL = Cfg()


def build(cfg: Cfg) -> bass.Bass:
    N, D, C, R, IB, JB = cfg.N, cfg.D, cfg.C, cfg.R, cfg.IB, cfg.JB
    JGRP, NG = cfg.JGRP, cfg.NG
    JW = 512               # matmul free-dim chunk
    NCW = R // JW          # free-dim chunks per output (2)

    nc = bacc.Bacc()

    x1T = nc.dram_tensor("x1T", [D, N], BF16, kind="ExternalInput")
    WT = nc.dram_tensor("WT", [D, D], BF16, kind="ExternalInput")
    adj = nc.dram_tensor("adj", [NG, 128, JGRP * R], FP8, kind="ExternalInput")
    qb = nc.dram_tensor("qb", [128, IB * D], F32, kind="ExternalInput")
    dinv_bc = nc.dram_tensor("dinv_bc", [128, R], BF16, kind="ExternalInput")
    out = nc.dram_tensor("out", [R, D], F32, kind="ExternalOutput")

    with tile.TileContext(nc, num_cores=C) as tc:
        const_p = tc.alloc_tile_pool(name="const", bufs=1)
        psA = tc.alloc_tile_pool(name="psA", bufs=1, space="PSUM")
        psB = tc.alloc_tile_pool(name="psB", bufs=5, space="PSUM")
        adj_p = tc.alloc_tile_pool(name="adjp", bufs=1)
        stage_p = tc.alloc_tile_pool(name="stage", bufs=2)

        # ---- inputs: ONE hwdge ring (sync), FIFO ordered by first use ---
        # WT first (gates every g-matmul), then x1T chunks, then the
        # adjacency stream (8MB, all resident: no slot waits), tail consts
        # last.
        WT_sb = const_p.tile([D, D], BF16, name="WT_sb")
        nc.sync.dma_start(out=WT_sb[:], in_=WT[:])
        x1T_sb = const_p.tile([D, N], BF16, name="x1T_sb")
        xw = N // cfg.XCH
        for q in range(cfg.XCH):
            nc.sync.dma_start(out=x1T_sb[:, q * xw:(q + 1) * xw],
                              in_=x1T[:, q * xw:(q + 1) * xw])
        at_tiles = []
        for jg in range(NG):
            at = adj_p.tile([128, JGRP * R], FP8, name=f"at{jg}")
            nc.sync.dma_start(out=at[:], in_=adj[jg, :, :])
            at_tiles.append(at)
        qb_sb = const_p.tile([128, IB * D], F32, name="qb_sb")
        nc.sync.dma_start(out=qb_sb[:], in_=qb[:])
        dinv_sb = const_p.tile([128, R], BF16, name="dinv_sb")
        nc.sync.dma_start(out=dinv_sb[:], in_=dinv_bc[:])
        ident = const_p.tile([128, 128], F32, name="ident")
        make_identity(nc, ident[:])

        # ---- PE pstate warmup: dummy matmuls on scratch tiles while the
        # input DMAs land (PE reaches full clock after ~3us of activity) --
        warm = const_p.tile([128, 256], BF16, name="warm")
        nc.vector.memset(warm[:], 0.0)
        for w in range(10):
            ps_w = psB.tile([128, 512], F32, name="ps_w", tag="ps")
            nc.tensor.matmul(ps_w[:, 0:128], lhsT=warm[:, 0:128],
                             rhs=warm[:, 128:256], start=True, stop=True)

        # ---- g = dinv * (x @ W.T), node-major blocks [j, d] -------------
        # g[jb*128+p, d] = sum_k x1T[k, jb*128+p] * WT[k, d]
        g_sb = const_p.tile([128, N], BF16, name="g_sb")
        for q in range(JB // 4):
            ps_g = psB.tile([128, 512], F32, name="ps_g", tag="ps")
            for t in range(4):
                jb = q * 4 + t
                nc.tensor.matmul(ps_g[:, t * D:(t + 1) * D],
                                 lhsT=x1T_sb[:, jb * 128:(jb + 1) * 128],
                                 rhs=WT_sb[:], start=True, stop=True)
            dst = g_sb[:, q * 512:(q + 1) * 512]
            if q % 2 == 0:
                nc.scalar.copy(dst, ps_g[:])
            else:
                nc.vector.tensor_copy(dst, ps_g[:])

        # ---- main GEMM: out2^T[d, i] += g_jb^T A[jb block, own cols] ----
        ps_out = psA.tile([128, R], F32, name="ps_out", tag="po")
        for jg in range(NG):
            at = at_tiles[jg]
            for t in range(JGRP):
                jb = jg * JGRP + t
                for c in range(NCW):
                    nc.tensor.matmul(
                        ps_out[:, c * JW:(c + 1) * JW],
                        lhsT=g_sb[:, jb * 128:(jb + 1) * 128],
                        rhs=at[:, t * R + c * JW:t * R + (c + 1) * JW],
                        start=(jb == 0), stop=(jb == JB - 1))

        # ---- tail: dinv scale (pre-transpose, broadcast multiplier),
        #      transpose, fused psum->sbuf + qb add, store ----------------
        o2 = stage_p.tile([128, R], F32, name="o2")
        o2r = stage_p.tile([128, JW], F32, name="o2r")
        nc.vector.tensor_tensor(o2[:, 0:JW], ps_out[:, 0:JW],
                                dinv_sb[:, 0:JW], Alu.mult)
        nc.scalar.copy(o2r[:], ps_out[:, JW:R])  # GPSIMD has no PSUM port
        nc.gpsimd.tensor_tensor(o2[:, JW:R], o2r[:],
                                dinv_sb[:, JW:R], Alu.mult)
        st = stage_p.tile([128, IB * D], F32, name="st")
        for b in range(IB):
            ps_t = psB.tile([128, 512], F32, name="ps_t", tag="ps")
            nc.tensor.transpose(ps_t[:, 0:128], o2[:, b * 128:(b + 1) * 128],
                                ident[:])
            sb = st[:, b * D:(b + 1) * D]
            nc.vector.tensor_tensor(sb, ps_t[:, 0:128],
                                    qb_sb[:, b * D:(b + 1) * D], Alu.add)
            eng = nc.scalar if b % 2 == 0 else nc.sync
            eng.dma_start(out=out[b * 128:(b + 1) * 128, :], in_=sb)

        for p in [stage_p, adj_p, psB, psA, const_p]:
            p.release()

    return nc


def make_in_maps(cfg: Cfg, x, edge_index, W, b_lin, bias):
    N, D, C, R, IB = cfg.N, cfg.D, cfg.C, cfg.R, cfg.IB
    x = np.asarray(x, dtype=np.float32)
    W = np.asarray(W, dtype=np.float32)
    b_lin = np.asarray(b_lin, dtype=np.float32)
    bias = np.asarray(bias, dtype=np.float32)
    ei = np.asarray(edge_index[0]).astype(np.int64)
    ej = np.asarray(edge_index[1]).astype(np.int64)

    # unique symmetric (dest, col) pairs == reference's at[].set collapse
    key = np.unique(np.concatenate([ei * N + ej, ej * N + ei]))
    dest = (key // N).astype(np.int64)
    col = (key % N).astype(np.int64)

    deg = np.bincount(dest, minlength=N).astype(np.float32)
    dinv = ((deg + np.float32(1e-6)) ** -0.5).astype(np.float32)
    # c_i = sum_j A_ij * dinv_j  (b_lin propagation constant)
    c = np.bincount(dest, weights=dinv[col].astype(np.float64),
                    minlength=N).astype(np.float32)

    one_fp8 = np.float32(1.0).astype(FP8_NP).view(np.uint8)
    A_u8 = np.zeros((N, N), np.uint8)
    A_u8[dest, col] = one_fp8

    x1T = np.ascontiguousarray((dinv[:, None] * x).T).astype(BF16_NP)
    WT = np.ascontiguousarray(W.T).astype(BF16_NP)

    in_maps = []
    for k in range(C):
        own = slice(k * R, (k + 1) * R)
        adj_k = (A_u8[:, own]
                 .reshape(cfg.NG, cfg.JGRP, 128, R)
                 .transpose(0, 2, 1, 3)
                 .reshape(cfg.NG, 128, cfg.JGRP * R))
        dinv_o = dinv[own].reshape(IB, 128)
        q = (dinv_o * c[own].reshape(IB, 128))              # [IB, 128]
        qb = (q[:, :, None] * b_lin[None, None, :]
              + bias[None, None, :]).astype(np.float32)     # [IB, 128, D]
        in_maps.append({
            "x1T": x1T,
            "WT": WT,
            "adj": np.ascontiguousarray(adj_k).view(FP8_NP),
            "qb": np.ascontiguousarray(
                qb.transpose(1, 0, 2).reshape(128, IB * D)),
            "dinv_bc": np.ascontiguousarray(np.broadcast_to(
                dinv[own].astype(BF16_NP)[None, :], (128, R))),
        })
    return in_maps


def kernel(x, edge_index, W, b_lin, bias, *, trace=False, cfg: Cfg = FULL):
    from concourse.bass_utils import run_bass_kernel_spmd

    if trace:
        _install_ntff_hook()
    in_maps = make_in_maps(cfg, x, edge_index, W, b_lin, bias)
    nc = build(cfg)
    nc.finalize()
    res = run_bass_kernel_spmd(nc, in_maps, core_ids=list(range(cfg.C)),
                               trace=trace)
    full = np.concatenate([r["out"] for r in res.results], axis=0)
    kernel.last_results = res
    return full.astype(np.float32)


kernel.last_results = None


def _install_ntff_hook():
    """Provide antenv.axon_hooks (missing on this image) so that
    run_bass_kernel_spmd(trace=True) can capture NTFF profiles via the
    axon ctypes hook from trn_agent_boot."""
    import sys as _sys
    import types

    try:
        import antenv.axon_hooks  # noqa: F401
        return True
    except ImportError:
        pass
    try:
        import antenv
        from trn_agent_boot.trn_boot import _ntff_profile_via_ctypes

        hook = _ntff_profile_via_ctypes("/opt/axon/libaxon_pjrt.so")
        mod = types.ModuleType("antenv.axon_hooks")
        mod.get_axon_ntff_profile_hook = lambda: hook
        mod.set_axon_ntff_profile_hook = lambda h: None
        _sys.modules["antenv.axon_hooks"] = mod
        antenv.axon_hooks = mod
        return hook is not None
    except Exception as e:  # profiling is best-effort
        print(f"ntff hook install failed: {e}", file=sys.stderr)
        return False
